# revision 2
# baseline (speedup 1.0000x reference)
"""GRU (B=512, T=512, I=32, H=64) + linear head via Picard sweeps, v2.

Data-parallel over 8 cores (BL=64 sequences each). Per core, batches are
processed in PAIRS sharing the 128 SBUF partitions: even batch of a pair
owns partitions 0:64, odd batch 64:128. All elementwise/activation work
is full-width [128, N], halving ACT/DVE time vs a 64-row layout, and the
gate output ordering is chosen so sigmoid results land directly in the
partitions the scan needs (no repartition DMAs, no DRAM staging).

Panel layout: col p*TP + t holds h_{t-1} for pair p (TP = T+1; col p*TP
is the pair's h_init = 0, written once by memset, never overwritten).
Per sweep, gates for a chunk of 4 pairs are built in PSUM: z and r from
h-matmuls (float32r, 1 cycle/col) with the x-part + biases accumulated
on top via bf16 matmuls from a resident x panel (x_even rows 0:32,
x_odd 32:64, ones row 64). Sigmoid evacuates PSUM full-width. The n
gate adds b_hh via scalar_tensor_tensor's per-partition scalar. The
blend recurrence h_t = z h_{t-1} - (z-1) n is solved exactly per pair
with tensor_tensor_scan (one scan per pair, 512 cols). Sweep 0 (h == 0)
needs only the x-side matmuls and also evacuates gxn for later sweeps.
The linear head runs in sweep 2 as each pair's states land.
"""

import numpy as np
import ml_dtypes

import concourse.bass as bass
import concourse.mybir as mybir
from concourse.tile import TileContext
from concourse.bass_utils import run_bass_kernel_spmd

B, T, I, O, H = 512, 512, 32, 16, 64
NCORES = 8
BL = B // NCORES            # 64 sequences per core
NP = BL // 2                # 32 pairs
TP = T + 1                  # padded panel columns per pair
PC = 4                      # pairs per chunk
CC = PC * T                 # 2048 gate columns per chunk
NCH = NP // PC              # 8 chunks
NSWEEPS = 4
f32 = mybir.dt.float32
f32r = mybir.dt.float32r
bf16 = mybir.dt.bfloat16
AF = mybir.ActivationFunctionType
ALU = mybir.AluOpType

X_NAME = "xq"


class _TC(TileContext):
    """TileContext whose instructions never carry >1 sync wait.

    This walrus build enforces a hard sync-wait-count limit per
    instruction; Tile's scheduler occasionally emits more (notably the
    kernel-tail drain). Split the excess onto same-engine nops inserted
    immediately before the offending instruction.
    """

    def _drain_and_barrier(self, tick_clock, wait_clock):
        super()._drain_and_barrier(tick_clock, wait_clock)
        nc = self.nc
        for fn in nc.m.functions:
            for blk in fn.blocks:
                out = []
                for inst in blk.instructions:
                    si = getattr(inst, "sync_info", None)
                    waits = list(si.on_wait) if si and si.on_wait else []
                    limit = 1
                    if len(waits) > limit:
                        si.on_wait = waits[-limit:]
                        extra = waits[:-limit]
                        for k in range(len(extra)):
                            eng = nc.engines[inst.engine]
                            nop = eng.nop(nofuse=True)
                            cur = nc.cur_bb.bb.instructions
                            assert cur and cur[-1] is nop.ins
                            cur.pop()
                            nop.ins.sync_info = mybir.SyncInfo(
                                on_wait=[extra[k]], on_update=[])
                            out.append(nop.ins)
                    out.append(inst)
                blk.instructions[:] = out


def build_bass(sweeps=NSWEEPS):
    nc = bass.Bass("TRN2", target_bir_lowering=False, debug=False,
                   num_devices=NCORES)
    xq_d = nc.dram_tensor("xq", [I * 2, NP * T], mybir.dt.int8,
                          kind="ExternalInput")
    sc_d = nc.dram_tensor("sc", [I * 2, 1], f32, kind="ExternalInput")
    whh_d = nc.dram_tensor("whh", [128, 3 * H], bf16, kind="ExternalInput")
    wx_d = nc.dram_tensor("wx", [I * 2 + 1, 3 * 2 * H + 48], bf16,
                          kind="ExternalInput")
    wlin_d = nc.dram_tensor("wlin", [128, O], bf16, kind="ExternalInput")
    bhn_d = nc.dram_tensor("bhn", [128, 1], f32, kind="ExternalInput")
    y_d = nc.dram_tensor("y", [2 * O, NP * T], mybir.dt.uint8,
                         kind="ExternalOutput")
    ysc_d = nc.dram_tensor("ysc", [2 * O, NCH], f32, kind="ExternalOutput")

    XR = I * 2 + 1            # 65 x-panel rows

    with _TC(nc) as tc:
        with (
            tc.tile_pool(name="const", bufs=1) as cp,
            tc.tile_pool(name="work", bufs=2) as wp,
        ):
            # state panel: col p*TP + t = h_{t-1}(pair p); bf16
            panel = cp.tile([128, NP * TP], bf16)
            nc.vector.memset(panel[:, :], 0.0)

            # resident x panel (bf16): rows 0:32 x_even, 32:64 x_odd, 64 ones
            # arrives int8 with per-feature-row scales; dequant on DVE
            xp = cp.tile([XR, NP * T], bf16)
            xq = cp.tile([I * 2, NP * T], mybir.dt.int8)
            nc.sync.dma_start(xq[:, 0:NP * T // 2], xq_d[:, 0:NP * T // 2])
            nc.sync.dma_start(xq[:, NP * T // 2:], xq_d[:, NP * T // 2:])
            sc = cp.tile([I * 2, 1], f32)
            nc.sync.dma_start(sc[:, :], sc_d[:, :])
            half = NP * T // 2
            nc.vector.tensor_scalar_mul(xp[0:I * 2, 0:half],
                                        xq[:, 0:half], sc[:, 0:1])
            nc.vector.tensor_scalar_mul(xp[0:I * 2, half:],
                                        xq[:, half:], sc[:, 0:1])
            nc.vector.memset(xp[I * 2:XR, :], 1.0)

            # stationary weights
            whh = cp.tile([128, 3 * H], bf16)      # cols: z | r | n
            nc.sync.dma_start(whh[:, :], whh_d[:, :])
            # cols: z | r | n x-blocks, then b_lin row block (row 64 only)
            wx = cp.tile([XR, 3 * 2 * H + 48], bf16)
            nc.sync.dma_start(wx[:, :], wx_d[:, :])
            wlin = cp.tile([128, O], bf16)
            nc.sync.dma_start(wlin[:, :], wlin_d[:, :])
            bhn = cp.tile([128, 1], f32)
            nc.sync.dma_start(bhn[:, :], bhn_d[:, :])

            # gxn persists across sweeps
            gxn = cp.tile([128, NP * T], bf16)
            # per-chunk per-channel y scales, exported once at the end
            yscs = cp.tile([48, NCH], f32)

            whz = whh[:, 0:H]
            whr = whh[:, H:2 * H]
            whn = whh[:, 2 * H:3 * H]
            wxz = wx[:, 0:2 * H]
            wxr = wx[:, 2 * H:4 * H]
            wxn = wx[:, 4 * H:6 * H]
            blin_s = wx[64:65, 6 * H:6 * H + 48]
            wlr = wlin

            with tc.tile_pool(name="ps", bufs=1, space="PSUM") as pp:
                for s in range(sweeps):
                    for c in range(NCH):
                        x0 = c * CC               # x/gate column base
                        psz = pp.tile([128, CC], f32, tag="psA", bufs=2,
                                      name=f"psz_{s}_{c}")
                        psr = pp.tile([128, CC], f32, tag="psA", bufs=2,
                                      name=f"psr_{s}_{c}")
                        for j in range(PC):
                            p = c * PC + j
                            bp = p * TP
                            lc = j * T
                            hv = panel[:, bp:bp + T]
                            if s > 0:
                                nc.tensor.matmul(
                                    psz[0:64, lc:lc + T], whz[0:64, :],
                                    hv[0:64, :], start=True, stop=False,
                                    skip_group_check=True)
                                nc.tensor.matmul(
                                    psz[64:128, lc:lc + T], whz[64:128, :],
                                    hv[64:128, :], start=True, stop=False,
                                    tile_position=(64, 64),
                                    skip_group_check=True)
                        for g in range(4):
                            nc.tensor.matmul(
                                psz[:, g * 512:(g + 1) * 512], wxz[:, :],
                                xp[:, x0 + g * 512:x0 + (g + 1) * 512],
                                start=(s == 0), stop=True,
                                skip_group_check=True)
                        for j in range(PC):
                            p = c * PC + j
                            bp = p * TP
                            lc = j * T
                            hv = panel[:, bp:bp + T]
                            if s > 0:
                                nc.tensor.matmul(
                                    psr[0:64, lc:lc + T], whr[0:64, :],
                                    hv[0:64, :], start=True, stop=False,
                                    skip_group_check=True)
                                nc.tensor.matmul(
                                    psr[64:128, lc:lc + T], whr[64:128, :],
                                    hv[64:128, :], start=True, stop=False,
                                    tile_position=(64, 64),
                                    skip_group_check=True)
                        for g in range(4):
                            nc.tensor.matmul(
                                psr[:, g * 512:(g + 1) * 512], wxr[:, :],
                                xp[:, x0 + g * 512:x0 + (g + 1) * 512],
                                start=(s == 0), stop=True,
                                skip_group_check=True)

                        zl = wp.tile([128, CC], bf16, tag="zl", bufs=2,
                                     name=f"zl_{s}_{c}")
                        nc.scalar.activation(zl[:, :], psz[:, :], AF.Sigmoid)
                        rt = wp.tile([128, CC], bf16, tag="rt", bufs=2,
                                     name=f"rt_{s}_{c}")
                        nc.scalar.activation(rt[:, :], psr[:, :], AF.Sigmoid)

                        m2 = wp.tile([128, CC], bf16, tag="m2", bufs=2,
                                     name=f"m2_{s}_{c}")
                        if s == 0:
                            # gxn into PSUM once; keep a copy for later sweeps
                            psn = pp.tile([128, CC], f32, tag="psA", bufs=2,
                                          name=f"psn_{s}_{c}")
                            for g in range(4):
                                nc.tensor.matmul(
                                    psn[:, g * 512:(g + 1) * 512],
                                    wxn[:, :],
                                    xp[:, x0 + g * 512:x0 + (g + 1) * 512],
                                    start=True, stop=True,
                                    skip_group_check=True)
                            nc.scalar.activation(gxn[:, x0:x0 + CC],
                                                 psn[:, :], AF.Copy)
                            # m2 = r*b_hhn + gxn
                            nc.vector.scalar_tensor_tensor(
                                m2[:, :], rt[:, :], bhn[:, 0:1], psn[:, :],
                                ALU.mult, ALU.add)
                        else:
                            psn = pp.tile([128, CC], f32, tag="psA", bufs=2,
                                          name=f"psn_{s}_{c}")
                            for j in range(PC):
                                p = c * PC + j
                                bp = p * TP
                                lc = j * T
                                hv = panel[:, bp:bp + T]
                                nc.tensor.matmul(
                                    psn[0:64, lc:lc + T], whn[0:64, :],
                                    hv[0:64, :], start=True, stop=True,
                                    skip_group_check=True)
                                nc.tensor.matmul(
                                    psn[64:128, lc:lc + T], whn[64:128, :],
                                    hv[64:128, :], start=True, stop=True,
                                    tile_position=(64, 64),
                                    skip_group_check=True)
                            # m = (ghn + b_hhn) * r
                            m = wp.tile([128, CC], bf16, tag="m", bufs=2,
                                        name=f"m_{s}_{c}")
                            nc.vector.scalar_tensor_tensor(
                                m[:, :], psn[:, :], bhn[:, 0:1], rt[:, :],
                                ALU.add, ALU.mult)
                            nc.vector.tensor_tensor(
                                m2[:, :], m[:, :], gxn[:, x0:x0 + CC],
                                ALU.add)
                        nt = wp.tile([128, CC], bf16, tag="nt", bufs=2,
                                     name=f"nt_{s}_{c}")
                        nc.scalar.activation(nt[:, :], m2[:, :], AF.Tanh)

                        zm1 = wp.tile([128, CC], bf16, tag="zm1", bufs=2,
                                      name=f"zm1_{s}_{c}")
                        nc.vector.tensor_scalar_sub(zm1[:, :], zl[:, :], 1.0)
                        ut = wp.tile([128, CC], bf16, tag="ut", bufs=2,
                                     name=f"ut_{s}_{c}")
                        nc.vector.tensor_tensor(ut[:, :], zm1[:, :],
                                                nt[:, :], ALU.mult)
                        for j in range(PC):
                            p = c * PC + j
                            bp = p * TP
                            lc = j * T
                            nc.vector.tensor_tensor_scan(
                                panel[:, bp + 1:bp + 1 + T],
                                zl[:, lc:lc + T], ut[:, lc:lc + T],
                                0.0, ALU.mult, ALU.subtract)

                        if s == sweeps - 1:
                            psy = pp.tile([48, CC], f32, tag="psA", bufs=2,
                                          name=f"psy_{c}")
                            for j in range(PC):
                                p = c * PC + j
                                bp = p * TP
                                lc = j * T
                                hv = panel[:, bp + 1:bp + 1 + T]
                                nc.tensor.matmul(
                                    psy[0:16, lc:lc + T], wlr[0:64, :],
                                    hv[0:64, :], start=True, stop=False,
                                    skip_group_check=True)
                                nc.tensor.matmul(
                                    psy[32:48, lc:lc + T], wlr[64:128, :],
                                    hv[64:128, :], start=True, stop=False,
                                    tile_position=(64, 32),
                                    skip_group_check=True)
                            # b_lin via the x-panel ones row
                            for g in range(4):
                                xs = xp[64:65,
                                        x0 + g * 512:x0 + (g + 1) * 512]
                                nc.tensor.matmul(
                                    psy[0:16, g * 512:(g + 1) * 512],
                                    blin_s[:, 0:16], xs,
                                    start=False, stop=True,
                                    tile_position=(64, 0),
                                    skip_group_check=True)
                                nc.tensor.matmul(
                                    psy[32:48, g * 512:(g + 1) * 512],
                                    blin_s[:, 32:48], xs,
                                    start=False, stop=True,
                                    tile_position=(64, 32),
                                    skip_group_check=True)
                            # int8 quantization straight from PSUM fp32
                            # with per-chunk per-channel scales (exported)
                            ym = wp.tile([48, 1], f32, tag="ym", bufs=2,
                                         name=f"ym_{c}")
                            nc.vector.tensor_reduce(
                                ym[0:16, 0:1], psy[0:16, :],
                                mybir.AxisListType.X, ALU.max,
                                apply_absolute_value=True)
                            nc.vector.tensor_reduce(
                                ym[32:48, 0:1], psy[32:48, :],
                                mybir.AxisListType.X, ALU.max,
                                apply_absolute_value=True)
                            nc.vector.tensor_scalar_max(
                                ym[0:16, 0:1], ym[0:16, 0:1], 1e-20)
                            nc.vector.tensor_scalar_max(
                                ym[32:48, 0:1], ym[32:48, 0:1], 1e-20)
                            ysc = yscs[:, c:c + 1]
                            nc.vector.tensor_scalar_mul(
                                ysc[0:16, 0:1], ym[0:16, 0:1], 1.0 / 127.0)
                            nc.vector.tensor_scalar_mul(
                                ysc[32:48, 0:1], ym[32:48, 0:1], 1.0 / 127.0)
                            yr = wp.tile([48, 1], f32, tag="yr", bufs=2,
                                         name=f"yr_{c}")
                            nc.vector.reciprocal(yr[0:16, 0:1],
                                                 ysc[0:16, 0:1])
                            nc.vector.reciprocal(yr[32:48, 0:1],
                                                 ysc[32:48, 0:1])
                            # uint8 offset encoding: trunc(v*rcp + 128.5)
                            # == round(v*rcp) + 128 (DVE truncates to int)
                            yq = wp.tile([48, CC], mybir.dt.uint8, tag="yq",
                                         bufs=2, name=f"yq_{c}")
                            nc.vector.tensor_scalar(
                                yq[0:16, :], psy[0:16, :], yr[0:16, 0:1],
                                128.5, ALU.mult, ALU.add)
                            nc.vector.tensor_scalar(
                                yq[32:48, :], psy[32:48, :], yr[32:48, 0:1],
                                128.5, ALU.mult, ALU.add)
                            nc.sync.dma_start(y_d[0:O, x0:x0 + CC],
                                              yq[0:16, :])
                            nc.sync.dma_start(y_d[O:2 * O, x0:x0 + CC],
                                              yq[32:48, :])
            nc.sync.dma_start(ysc_d[0:O, :], yscs[0:16, :])
            nc.sync.dma_start(ysc_d[O:2 * O, :], yscs[32:48, :])
    return nc


def prep_consts(W_ih, W_hh, b_ih, b_hh, W_lin, b_lin):
    W_ih = np.asarray(W_ih, np.float32)
    W_hh = np.asarray(W_hh, np.float32)
    b_ih = np.asarray(b_ih, np.float32)
    b_hh = np.asarray(b_hh, np.float32)
    W_lin = np.asarray(W_lin, np.float32)
    b_lin = np.asarray(b_lin, np.float32)
    bf = ml_dtypes.bfloat16

    # whh [128, 3H]: gate order z | r | n; both partition halves identical
    whh = np.zeros((128, 3 * H), np.float32)
    for gi, sl in enumerate((slice(H, 2 * H), slice(0, H),
                             slice(2 * H, 3 * H))):
        wt = W_hh[sl].T                     # [H, H]
        whh[0:64, gi * H:(gi + 1) * H] = wt
        whh[64:128, gi * H:(gi + 1) * H] = wt

    # wx [65, 6H+48]: per gate a [65, 128] block: rows 0:32 -> even outputs
    # 0:64, rows 32:64 -> odd outputs 64:128, row 64 = bias for both;
    # then a [65, 48] b_lin block (row 64 only) for the linear-head bias
    wx = np.zeros((2 * I + 1, 3 * 2 * H + 48), np.float32)
    gates = ((slice(H, 2 * H), True), (slice(0, H), True),
             (slice(2 * H, 3 * H), False))
    for gi, (sl, with_bhh) in enumerate(gates):
        blk = np.zeros((2 * I + 1, 2 * H), np.float32)
        wt = W_ih[sl].T                     # [I, H]
        blk[0:I, 0:H] = wt
        blk[I:2 * I, H:2 * H] = wt
        bias = b_ih[sl] + (b_hh[sl] if with_bhh else 0.0)
        blk[2 * I, 0:H] = bias
        blk[2 * I, H:2 * H] = bias
        wx[:, gi * 2 * H:(gi + 1) * 2 * H] = blk
    wx[2 * I, 6 * H:6 * H + O] = b_lin
    wx[2 * I, 6 * H + 32:6 * H + 48] = b_lin

    wlin_t = np.zeros((128, O), np.float32)
    wlin_t[0:64] = W_lin.T
    wlin_t[64:128] = W_lin.T

    bhn = np.zeros((128, 1), np.float32)
    bhn[0:64, 0] = b_hh[2 * H:3 * H]
    bhn[64:128, 0] = b_hh[2 * H:3 * H]

    return {
        "whh": np.ascontiguousarray(whh.astype(bf)),
        "wx": np.ascontiguousarray(wx.astype(bf)),
        "wlin": np.ascontiguousarray(wlin_t.astype(bf)),
        "bhn": bhn,
    }


def prep_x(x_shard):
    """[BL, T, I] float -> int8 pair panel [64, NP*T] + fp32 row scales."""
    x_shard = np.asarray(x_shard, np.float32)
    xr = np.empty((2 * I, NP, T), np.float32)
    xr[0:I] = x_shard[0::2].transpose(2, 0, 1)
    xr[I:2 * I] = x_shard[1::2].transpose(2, 0, 1)
    xr = xr.reshape(2 * I, NP * T)
    sc = np.abs(xr).max(axis=1, keepdims=True) / 127.0
    sc = np.maximum(sc, 1e-30)
    xq = np.clip(np.round(xr / sc), -127, 127).astype(np.int8)
    return np.ascontiguousarray(xq), sc.astype(np.float32)


def postprocess_y(yt, ysc=None):
    """[2O, NP*T] (int8 + scales or float) -> [BL, T, O]."""
    yt = np.asarray(yt, np.float32)
    if ysc is not None:
        # yt uint8 offset-128 values, ysc [2O, NCH]: per-chunk scales
        yt = (yt.reshape(2 * O, NCH, CC) - 128.0) * np.asarray(
            ysc, np.float32)[:, :, None]
        yt = yt.reshape(2 * O, NP * T)
    return yt.reshape(2, O, NP, T).transpose(2, 0, 3, 1).reshape(BL, T, O)


_cached = {}


def kernel(x, W_ih, W_hh, b_ih, b_hh, W_lin, b_lin):
    x = np.asarray(x, np.float32)
    consts = prep_consts(W_ih, W_hh, b_ih, b_hh, W_lin, b_lin)
    if "nc" not in _cached:
        _cached["nc"] = build_bass()
    nc = _cached["nc"]
    in_maps = []
    for cid in range(NCORES):
        m = dict(consts)
        m["xq"], m["sc"] = prep_x(x[cid * BL:(cid + 1) * BL])
        in_maps.append(m)
    _cached["in_maps"] = in_maps
    res = run_bass_kernel_spmd(nc, in_maps, core_ids=list(range(NCORES)))
    outs = []
    for cid in range(NCORES):
        yt = np.asarray(res.results[cid]["y"])
        ysc = np.asarray(res.results[cid]["ysc"])
        outs.append(postprocess_y(yt, ysc))
    return np.concatenate(outs, 0)
